# revision 17
# baseline (speedup 1.0000x reference)
"""Trainium2 Bass kernel for nn_AttnDBGNNLayer (8-core SPMD).

kernel(**inputs) takes the FULL inputs (as produced by setup_inputs) and
returns the FULL output (new_A, new_B), distributing across 8 NeuronCores.
"""
import sys

if "/opt/trn_rl_repo" not in sys.path:
    sys.path.insert(0, "/opt/trn_rl_repo")

import numpy as np
import ml_dtypes

import concourse.bacc as bacc
import concourse.tile as tile
import concourse.mybir as mybir
from concourse import bass_utils

BF16 = ml_dtypes.bfloat16

N = 8192          # nodes per type
D = 128           # feature dim
NCORES = 8
R = N // NCORES   # rows (q / dst nodes) per core = 1024
QG = 512          # q-group width
KB = N // 128     # number of 128-wide k blocks = 64
NBLK = R // 128   # dst blocks per core = 8
SCALE = 1.0 / np.sqrt(np.float32(D))

F32 = mybir.dt.float32
BF = mybir.dt.bfloat16
I16 = mybir.dt.int16


# ---------------------------------------------------------------- host prep

def _wrap_idx(flat):
    """[n] int -> [128, n/16] int16 wrapped layout for dma_gather
    (16-partition wrap, replicated across the 8 Q7 cores)."""
    flat = np.asarray(flat, np.int16)
    assert flat.size % 16 == 0
    return np.ascontiguousarray(np.tile(flat.reshape(-1, 16).T, (8, 1)))


def _edge_prep(ei):
    """Partition edges by dst core, bucket by dst block.

    Returns per-core lists of (per-block lists of (src, dstoff)) plus the
    global max chunk count M_B, and per-core degree vectors [R].
    """
    src = np.asarray(ei[0], np.int64)
    dst = np.asarray(ei[1], np.int64)
    per_core = []
    degs = []
    maxch = 1
    for c in range(NCORES):
        sel = (dst >> 10) == c
        s_c = src[sel]
        d_c = dst[sel] - c * R
        deg = np.bincount(d_c, minlength=R).astype(np.float32)
        degs.append(deg)
        blocks = []
        for b in range(NBLK):
            bsel = (d_c >> 7) == b
            sb = s_c[bsel]
            db = d_c[bsel] - b * 128          # 0..127 offset inside block
            blocks.append((sb, db))
            maxch = max(maxch, -(-sb.size // 128))
        per_core.append(blocks)
    return per_core, degs, maxch


def _edge_arrays(per_core, M):
    """Pad each (core, block) bucket to M*128 edges; emit wrapped int16
    src-index and dstoff-index arrays of shape [16, NBLK*M*8] per core."""
    out = []
    for c in range(NCORES):
        srcs = []
        offs = []
        for b in range(NBLK):
            sb, db = per_core[c][b]
            n = sb.size
            pad = M * 128 - n
            srcs.append(np.concatenate([sb, np.zeros(pad, np.int64)]))
            # one-hot table row 128 is all-zero -> padded edges contribute 0
            offs.append(np.concatenate([db, np.full(pad, 128, np.int64)]))
        out.append((_wrap_idx(np.concatenate(srcs)),
                    _wrap_idx(np.concatenate(offs))))
    return out


# ---------------------------------------------------------------- program

_PROG_CACHE = {}


def build_program(M, dbg=False, stage=3):
    """M: dict graph -> chunk count per dst block. Same program on all cores.
    stage: 1=attention+tables+allgather only, 2=+gathers, 3=full."""
    nc = bacc.Bacc("TRN2", target_bir_lowering=False, debug=False,
                   num_devices=NCORES)

    G = ("AB", "BA", "AA")

    # ---- I/O declarations (per-core data, identical shapes on all cores)
    x0t = {t: nc.dram_tensor(f"x0t_{t}", [128, N], BF, kind="ExternalInput")
           for t in "AB"}
    x0q = {t: nc.dram_tensor(f"x0q_{t}", [128, R], BF, kind="ExternalInput")
           for t in "AB"}
    wqT = {t: nc.dram_tensor(f"wqT_{t}", [128, 128], BF, kind="ExternalInput")
           for t in "AB"}
    wkT = {t: nc.dram_tensor(f"wkT_{t}", [128, 128], BF, kind="ExternalInput")
           for t in "AB"}
    wvoT = {t: nc.dram_tensor(f"wvoT_{t}", [128, 128], BF, kind="ExternalInput")
            for t in "AB"}
    bqs = {t: nc.dram_tensor(f"bqs_{t}", [128, 1], F32, kind="ExternalInput")
           for t in "AB"}
    bk = {t: nc.dram_tensor(f"bk_{t}", [128, 1], F32, kind="ExternalInput")
          for t in "AB"}
    eye_tab = nc.dram_tensor("eye_tab", [129, 128], BF, kind="ExternalInput")
    wlT = {g: nc.dram_tensor(f"wlT_{g}", [128, 128], BF, kind="ExternalInput")
           for g in G}
    wrT = {t: nc.dram_tensor(f"wrT_{t}", [128, 128], BF, kind="ExternalInput")
           for t in "AB"}
    c0 = {t: nc.dram_tensor(f"c0_{t}", [1, 128], F32, kind="ExternalInput")
          for t in "AB"}
    c1 = {g: nc.dram_tensor(f"c1_{g}", [1, 128], F32, kind="ExternalInput")
          for g in G}
    deg = {g: nc.dram_tensor(f"deg_{g}", [1, R], F32, kind="ExternalInput")
           for g in G}
    sidx = {g: nc.dram_tensor(f"sidx_{g}", [128, NBLK * M[g] * 8], I16,
                              kind="ExternalInput") for g in G}
    oidx = {g: nc.dram_tensor(f"oidx_{g}", [128, NBLK * M[g] * 8], I16,
                              kind="ExternalInput") for g in G}
    out_d = {t: nc.dram_tensor(f"out_{t}", [R, 128], F32, kind="ExternalOutput")
             for t in "AB"}
    dbg_d = {}
    if dbg:
        for t in "AB":
            dbg_d[f"ht_{t}"] = nc.dram_tensor(f"dbg_ht_{t}", [128, R], BF,
                                              kind="ExternalOutput")
        for g in G:
            dbg_d[f"tab_{g}"] = nc.dram_tensor(f"dbg_tab_{g}", [N, 128], BF,
                                               kind="ExternalOutput")

    # internal DRAM for collectives
    tab_loc = {g: nc.dram_tensor(f"tabloc_{g}", [R, 128], BF) for g in G}
    tab = {g: nc.dram_tensor(f"tab_{g}", [N, 128], BF, addr_space="Shared")
           for g in G}

    src_type = {"AB": "A", "BA": "B", "AA": "A"}
    graphs_of = {"A": ("BA", "AA"), "B": ("AB",)}   # inbound graphs per output
    tables_of = {"A": ("AB", "AA"), "B": ("BA",)}   # tables whose src is t

    with tile.TileContext(nc) as tc:
        with (
            tc.tile_pool(name="const", bufs=1) as cp,
            tc.tile_pool(name="big", bufs=1) as bp,
            tc.tile_pool(name="pt", bufs=3) as ptp,
            tc.tile_pool(name="gAB", bufs=3) as gpAB,
            tc.tile_pool(name="gBA", bufs=1) as gpBA,
            tc.tile_pool(name="gAA", bufs=1) as gpAA,
            tc.tile_pool(name="ps_s", bufs=2, space="PSUM") as ps_s,
            tc.tile_pool(name="ps_u", bufs=2, space="PSUM") as ps_u,
            tc.tile_pool(name="ps_sm", bufs=1, space="PSUM") as ps_sm,
        ):
            gp = {"AB": gpAB, "BA": gpBA, "AA": gpAA}

            # ---------------- constants into SBUF
            def load_const(dram, shape, dt):
                t = cp.tile(shape, dt, tag=dram.name)
                nc.sync.dma_start(out=t[:], in_=dram[:])
                return t

            wqT_s = {t: load_const(wqT[t], [128, 128], BF) for t in "AB"}
            wkT_s = {t: load_const(wkT[t], [128, 128], BF) for t in "AB"}
            wvoT_s = {t: load_const(wvoT[t], [128, 128], BF) for t in "AB"}
            bqs_s = {t: load_const(bqs[t], [128, 1], F32) for t in "AB"}
            bk_s = {t: load_const(bk[t], [128, 1], F32) for t in "AB"}
            wlT_s = {g: load_const(wlT[g], [128, 128], BF) for g in G}
            wrT_s = {t: load_const(wrT[t], [128, 128], BF) for t in "AB"}
            c0_s = {t: load_const(c0[t], [1, 128], F32) for t in "AB"}
            c1_s = {g: load_const(c1[g], [1, 128], F32) for g in G}
            deg_s = {g: load_const(deg[g], [1, R], F32) for g in G}

            ident = cp.tile([128, 128], BF, tag="ident")
            from concourse.masks import make_identity
            make_identity(nc, ident[:])
            ones_col = cp.tile([128, 1], BF, tag="ones_col")
            nc.vector.memset(ones_col[:], 1.0)
            ones_row = cp.tile([1, 128], F32, tag="ones_row")
            nc.vector.memset(ones_row[:], 1.0)

            ht = {t: bp.tile([128, R], BF, tag=f"ht_{t}", name=f"ht_{t}")
                  for t in "AB"}

            gtiles = {}

            def emit_gather(g, b):
                m_g = M[g]
                isl = slice(b * m_g * 8, (b + 1) * m_g * 8)
                si = gp[g].tile([128, m_g * 8], I16, tag=f"si_{g}",
                                name=f"si_{g}_{b}")
                oi = gp[g].tile([128, m_g * 8], I16, tag=f"oi_{g}",
                                name=f"oi_{g}_{b}")
                nc.sync.dma_start(out=si[:], in_=sidx[g][:, isl])
                nc.sync.dma_start(out=oi[:], in_=oidx[g][:, isl])
                msg = gp[g].tile([128, m_g, 128], BF, tag=f"m_{g}",
                                 name=f"msg_{g}_{b}")
                poh = gp[g].tile([128, m_g, 128], BF, tag=f"p_{g}",
                                 name=f"poh_{g}_{b}")
                nc.gpsimd.dma_gather(
                    out_ap=msg[:], in_ap=tab[g][:], idxs_ap=si[:],
                    num_idxs=m_g * 128, num_idxs_reg=m_g * 128, elem_size=128,
                    single_packet=False)
                nc.gpsimd.dma_gather(
                    out_ap=poh[:], in_ap=eye_tab[:], idxs_ap=oi[:],
                    num_idxs=m_g * 128, num_idxs_reg=m_g * 128, elem_size=128,
                    single_packet=False)
                gtiles[(g, b)] = (msg, poh)

            # ---------------- attention per type
            for t in "AB":
                x0_s = bp.tile([128, N], BF, tag="x0t")
                nc.sync.dma_start(out=x0_s[:], in_=x0t[t][:])
                x0q_s = bp.tile([128, R], BF, tag="x0q")
                nc.sync.dma_start(out=x0q_s[:], in_=x0q[t][:])

                kt = bp.tile([128, N], BF, tag="kt")
                vt = bp.tile([128, N], BF, tag="vt")
                qt = bp.tile([128, R], BF, tag="qt")

                # Kt = Wk @ x0T (+bk), feature-major
                for j in range(N // 512):
                    ps = ps_s.tile([128, 1024], F32, tag="sc")
                    nc.tensor.matmul(ps[:, :512], lhsT=wkT_s[t][:],
                                     rhs=x0_s[:, j * 512:(j + 1) * 512],
                                     start=True, stop=True)
                    nc.scalar.activation(kt[:, j * 512:(j + 1) * 512],
                                         ps[:, :512],
                                         mybir.ActivationFunctionType.Identity,
                                         bias=bk_s[t][:, :])
                # Qt = s*(Wq @ x0T_own + bq)
                for j in range(R // 512):
                    ps = ps_s.tile([128, 1024], F32, tag="sc")
                    nc.tensor.matmul(ps[:, :512], lhsT=wqT_s[t][:],
                                     rhs=x0q_s[:, j * 512:(j + 1) * 512],
                                     start=True, stop=True)
                    nc.scalar.activation(qt[:, j * 512:(j + 1) * 512],
                                         ps[:, :512],
                                         mybir.ActivationFunctionType.Identity,
                                         bias=bqs_s[t][:, :], scale=float(SCALE))
                # V~ = x0 @ (Wout Wv)^T, node-major blocks (4 per psum tile)
                for vg in range(KB // 4):
                    ps = ps_u.tile([128, 512], F32, tag="ut")
                    for i in range(4):
                        nb = vg * 4 + i
                        nc.tensor.matmul(ps[:, i * 128:(i + 1) * 128],
                                         lhsT=x0_s[:, nb * 128:(nb + 1) * 128],
                                         rhs=wvoT_s[t][:], start=True, stop=True)
                    nc.vector.tensor_copy(vt[:, vg * 512:(vg + 1) * 512], ps[:])

                # attention q-groups
                for qg in range(R // QG):
                    q_sl = slice(qg * QG, (qg + 1) * QG)
                    ut_ps = ps_u.tile([128, QG], F32, tag="ut")
                    racc0 = bp.tile([128, QG], BF, tag="racc0")
                    racc1 = bp.tile([128, QG], BF, tag="racc1")
                    nc.vector.memset(racc0[:], 0.0)
                    nc.vector.memset(racc1[:], 0.0)
                    for pr in range(KB // 2):
                        kb0 = 2 * pr
                        sc = ps_s.tile([128, 1024], F32, tag="sc")
                        nc.tensor.matmul(sc[:, :512],
                                         lhsT=kt[:, kb0 * 128:(kb0 + 1) * 128],
                                         rhs=qt[:, q_sl], start=True, stop=True)
                        nc.tensor.matmul(sc[:, 512:],
                                         lhsT=kt[:, (kb0 + 1) * 128:(kb0 + 2) * 128],
                                         rhs=qt[:, q_sl], start=True, stop=True)
                        pt = ptp.tile([128, 1024], BF, tag="pt")
                        nc.scalar.activation(pt[:], sc[:],
                                             mybir.ActivationFunctionType.Exp)
                        nc.tensor.matmul(ut_ps[:],
                                         lhsT=vt[:, kb0 * 128:(kb0 + 1) * 128],
                                         rhs=pt[:, :512],
                                         start=(pr == 0), stop=False)
                        nc.tensor.matmul(ut_ps[:],
                                         lhsT=vt[:, (kb0 + 1) * 128:(kb0 + 2) * 128],
                                         rhs=pt[:, 512:],
                                         start=False, stop=(pr == KB // 2 - 1))
                        nc.vector.tensor_add(racc0[:], racc0[:], pt[:, :512])
                        nc.vector.tensor_add(racc1[:], racc1[:], pt[:, 512:])

                    ut_sb = bp.tile([128, QG], BF, tag="ut_sb")
                    nc.vector.tensor_copy(ut_sb[:], ut_ps[:])
                    for sub in range(QG // 128):
                        s_sl = slice(sub * 128, (sub + 1) * 128)
                        rp = ps_sm.tile([128, 512], F32, tag="sm")
                        nc.tensor.matmul(rp[:, :1], lhsT=racc0[:, s_sl],
                                         rhs=ones_col[:], start=True, stop=False)
                        nc.tensor.matmul(rp[:, :1], lhsT=racc1[:, s_sl],
                                         rhs=ones_col[:], start=False, stop=True)
                        rinv = bp.tile([128, 1], F32, tag="rinv")
                        nc.vector.reciprocal(rinv[:], rp[:, :1])
                        # transpose Ut tile -> [q, f], scale by 1/r
                        tp = ps_sm.tile([128, 512], BF, tag="smb")
                        nc.tensor.transpose(tp[:, :128], ut_sb[:, s_sl],
                                            ident[:])
                        hn = bp.tile([128, 128], BF, tag="hn")
                        nc.scalar.activation(hn[:], tp[:, :128],
                                             mybir.ActivationFunctionType.Copy,
                                             scale=rinv[:, :])
                        # transpose back -> feature-major ht
                        tp2 = ps_sm.tile([128, 512], BF, tag="smb")
                        nc.tensor.transpose(tp2[:, :128], hn[:], ident[:])
                        nc.vector.tensor_copy(
                            ht[t][:, qg * QG + sub * 128: qg * QG + (sub + 1) * 128],
                            tp2[:, :128])

                # tables with src == t: tab_loc_g = (h @ wl_g^T) node-major
                for g in tables_of[t]:
                    tsb = bp.tile([128, NBLK * 128], BF, tag=f"tsb_{g}")
                    for nb in range(NBLK):
                        ps = ps_sm.tile([128, 512], F32, tag="sm")
                        nc.tensor.matmul(ps[:, :128],
                                         lhsT=ht[t][:, nb * 128:(nb + 1) * 128],
                                         rhs=wlT_s[g][:], start=True, stop=True)
                        nc.vector.tensor_copy(tsb[:, nb * 128:(nb + 1) * 128],
                                              ps[:, :128])
                    for nb in range(NBLK):
                        nc.sync.dma_start(
                            out=tab_loc[g][nb * 128:(nb + 1) * 128, :],
                            in_=tsb[:, nb * 128:(nb + 1) * 128])
                    nc.gpsimd.collective_compute(
                        "AllGather", mybir.AluOpType.bypass,
                        replica_groups=[list(range(NCORES))],
                        ins=[tab_loc[g].ap()], outs=[tab[g].ap()])

                if t == "A" and stage >= 2:
                    # prefetch out_B's gathers so they overlap B's attention
                    for b in range(NBLK):
                        emit_gather("AB", b)

            if dbg:
                for t in "AB":
                    nc.sync.dma_start(out=dbg_d[f"ht_{t}"][:], in_=ht[t][:])
                for g in G:
                    nc.sync.dma_start(out=dbg_d[f"tab_{g}"][:], in_=tab[g][:])

            # ---------------- phase 2: gather + segment-sum + linears
            for t in ("BA" if stage >= 3 else ""):
                for b in range(NBLK):
                    po = ps_u.tile([128, QG], F32, tag="ut")
                    first = True
                    for g in graphs_of[t]:
                        if (g, b) not in gtiles:
                            emit_gather(g, b)
                        msg, poh = gtiles.pop((g, b))
                        for m in range(M[g]):
                            nc.tensor.matmul(po[:, :128], lhsT=poh[:, m, :],
                                             rhs=msg[:, m, :],
                                             start=first, stop=False)
                            first = False
                        # deg ⊗ c1 rank-1 correction
                        nc.tensor.matmul(po[:, :128],
                                         lhsT=deg_s[g][:, b * 128:(b + 1) * 128],
                                         rhs=c1_s[g][:], start=False, stop=False)
                    # lin_r
                    nc.tensor.matmul(po[:, :128],
                                     lhsT=ht[t][:, b * 128:(b + 1) * 128],
                                     rhs=wrT_s[t][:], start=False, stop=False)
                    # constant row c0
                    nc.tensor.matmul(po[:, :128], lhsT=ones_row[:],
                                     rhs=c0_s[t][:], start=False, stop=True)
                    osb = bp.tile([128, 128], F32, tag="osb")
                    nc.vector.tensor_copy(osb[:], po[:, :128])
                    nc.sync.dma_start(out=out_d[t][b * 128:(b + 1) * 128, :],
                                      in_=osb[:])

    nc.compile()
    return nc


# ---------------------------------------------------------------- kernel

def _prep(inputs, dbg=False):
    ins = {k: np.asarray(v) for k, v in inputs.items()}
    G = ("AB", "BA", "AA")
    per_core_edges = {}
    degs = {}
    M = {}
    for g in G:
        pc, dg, mx = _edge_prep(ins[f"ei_{g}"])
        arr = _edge_arrays(pc, mx)
        per_core_edges[g] = arr
        degs[g] = dg
        M[g] = mx

    # host-folded weights / constants
    def bf(x):
        return np.ascontiguousarray(np.asarray(x, np.float32)).astype(BF16)

    com = {}
    for t in "AB":
        iw = ins[f"inW_{t}"].astype(np.float32)
        ib = ins[f"inB_{t}"].astype(np.float32)
        ow = ins[f"outW_{t}"].astype(np.float32)
        ob = ins[f"outB_{t}"].astype(np.float32)
        com[f"wqT_{t}"] = bf(iw[0:128].T)
        com[f"wkT_{t}"] = bf(iw[128:256].T)
        wvo = ow @ iw[256:384]
        com[f"wvoT_{t}"] = bf(wvo.T)
        com[f"bqs_{t}"] = (ib[0:128] * SCALE).reshape(128, 1).astype(np.float32)
        com[f"bk_{t}"] = ib[128:256].reshape(128, 1).astype(np.float32)
        com[f"bout_eff_{t}"] = ow @ ib[256:384] + ob
    for g in G:
        com[f"wlT_{g}"] = bf(ins[f"wl_{g}"].T)
        com[f"c1_{g}"] = (ins[f"wl_{g}"].astype(np.float32)
                          @ com[f"bout_eff_{src_type_of(g)}"]).reshape(1, 128)
    com["wrT_B"] = bf(ins["wr_AB"].T)
    com["wrT_A"] = bf((ins["wr_BA"] + ins["wr_AA"]).T)
    com["c0_B"] = (ins["bl_AB"].astype(np.float32)
                   + ins["wr_AB"].astype(np.float32) @ com["bout_eff_B"]
                   ).reshape(1, 128)
    com["c0_A"] = (ins["bl_BA"].astype(np.float32)
                   + ins["bl_AA"].astype(np.float32)
                   + (ins["wr_BA"] + ins["wr_AA"]).astype(np.float32)
                   @ com["bout_eff_A"]).reshape(1, 128)
    eye = np.zeros((129, 128), np.float32)
    eye[:128, :128] = np.eye(128)
    com["eye_tab"] = eye.astype(BF16)

    x0T = {t: np.ascontiguousarray(
        ins[f"x_{t}"][:, 0, :].astype(np.float32).T).astype(BF16) for t in "AB"}

    in_maps = []
    for c in range(NCORES):
        m = {}
        for t in "AB":
            m[f"x0t_{t}"] = x0T[t]
            m[f"x0q_{t}"] = np.ascontiguousarray(
                x0T[t][:, c * R:(c + 1) * R])
            for k in ("wqT", "wkT", "wvoT", "bqs", "bk", "wrT", "c0"):
                m[f"{k}_{t}"] = com[f"{k}_{t}"]
        m["eye_tab"] = com["eye_tab"]
        for g in G:
            m[f"wlT_{g}"] = com[f"wlT_{g}"]
            m[f"c1_{g}"] = com[f"c1_{g}"].astype(np.float32)
            m[f"deg_{g}"] = degs[g][c].reshape(1, R)
            m[f"sidx_{g}"] = per_core_edges[g][c][0]
            m[f"oidx_{g}"] = per_core_edges[g][c][1]
        in_maps.append(m)
    return in_maps, M


def src_type_of(g):
    return {"AB": "A", "BA": "B", "AA": "A"}[g]


def kernel(**inputs):
    in_maps, M = _prep(inputs)
    key = tuple(sorted(M.items()))
    if key not in _PROG_CACHE:
        _PROG_CACHE[key] = build_program(M)
    nc = _PROG_CACHE[key]
    res = bass_utils.run_bass_kernel_spmd(
        nc, in_maps, core_ids=list(range(NCORES)))
    x_A = np.asarray(inputs["x_A"], np.float32)
    x_B = np.asarray(inputs["x_B"], np.float32)
    new_A = x_A.copy()
    new_B = x_B.copy()
    for c in range(NCORES):
        new_A[c * R:(c + 1) * R, 0, :] = res.results[c]["out_A"]
        new_B[c * R:(c + 1) * R, 0, :] = res.results[c]["out_B"]
    return new_A, new_B


# revision 20
# speedup vs baseline: 3.8363x; 3.8363x over previous
"""Trainium2 Bass kernel for nn_AttnDBGNNLayer (8-core SPMD).

kernel(**inputs) takes the FULL inputs (as produced by setup_inputs) and
returns the FULL output (new_A, new_B), distributing across 8 NeuronCores.

Design:
- q-rows of both attentions sharded 8-way (1024 rows/core); K/V computed
  replicated from a feature-major x0^T; single-pass unnormalized softmax
  (scores are tiny, no max subtraction); out-projection folded into V
  (Wvo = Wout @ Wv); row-sum via DVE accumulation + ones-matmul; normalize
  via PE-transpose + per-partition scale.
- per-graph gather tables hold h @ wl^T (lin_l folded), AllGathered in bf16.
- message aggregation as dense count-matrix matmuls: out^T += tab_g^T @ C_g
  with C_g the per-core [8192 src, 1024 dst] edge-count matrix in fp8
  (counts are small ints, exact); lin_r / biases / degree corrections are
  folded in as K=1 matmuls into the same PSUM accumulation.
- outputs are produced feature-major and transposed on the host.
"""
import sys

if "/opt/trn_rl_repo" not in sys.path:
    sys.path.insert(0, "/opt/trn_rl_repo")

import numpy as np
import ml_dtypes

import concourse.bacc as bacc
import concourse.tile as tile
import concourse.mybir as mybir
from concourse import bass_utils

BF16 = ml_dtypes.bfloat16
FP8 = ml_dtypes.float8_e4m3

N = 8192          # nodes per type
D = 128           # feature dim
NCORES = 8
R = N // NCORES   # rows (q / dst nodes) per core = 1024
QG = 512          # q-group width
KB = N // 128     # number of 128-wide k blocks = 64
NBLK = R // 128   # dst blocks per core = 8
SCALE = 1.0 / np.sqrt(np.float32(D))

F32 = mybir.dt.float32
BF = mybir.dt.bfloat16
F8 = mybir.dt.float8e4

G = ("AB", "BA", "AA")
SRC_T = {"AB": "A", "BA": "B", "AA": "A"}
GRAPHS_OF = {"A": ("BA", "AA"), "B": ("AB",)}   # inbound graphs per output
TABLES_OF = {"A": ("AB", "AA"), "B": ("BA",)}   # tables whose src is t

_PROG_CACHE = {}


def build_program(dbg=False, stage=3):
    nc = bacc.Bacc("TRN2", target_bir_lowering=False, debug=False,
                   num_devices=NCORES)

    # ---- I/O (identical shapes on all cores; per-core data)
    x0t = {t: nc.dram_tensor(f"x0t_{t}", [128, N], BF, kind="ExternalInput")
           for t in "AB"}
    x0q = {t: nc.dram_tensor(f"x0q_{t}", [128, R], BF, kind="ExternalInput")
           for t in "AB"}
    wqT = {t: nc.dram_tensor(f"wqT_{t}", [128, 128], BF, kind="ExternalInput")
           for t in "AB"}
    wkT = {t: nc.dram_tensor(f"wkT_{t}", [128, 128], BF, kind="ExternalInput")
           for t in "AB"}
    wvoT = {t: nc.dram_tensor(f"wvoT_{t}", [128, 128], BF, kind="ExternalInput")
            for t in "AB"}
    bqs = {t: nc.dram_tensor(f"bqs_{t}", [128, 1], F32, kind="ExternalInput")
           for t in "AB"}
    bk = {t: nc.dram_tensor(f"bk_{t}", [128, 1], F32, kind="ExternalInput")
          for t in "AB"}
    wlT = {g: nc.dram_tensor(f"wlT_{g}", [128, 128], BF, kind="ExternalInput")
           for g in G}
    wrT = {t: nc.dram_tensor(f"wrT_{t}", [128, 128], BF, kind="ExternalInput")
           for t in "AB"}
    c0 = {t: nc.dram_tensor(f"c0_{t}", [1, 128], F32, kind="ExternalInput")
          for t in "AB"}
    c1 = {g: nc.dram_tensor(f"c1_{g}", [1, 128], F32, kind="ExternalInput")
          for g in G}
    deg = {g: nc.dram_tensor(f"deg_{g}", [1, R], F32, kind="ExternalInput")
           for g in G}
    ct = {g: nc.dram_tensor(f"ct_{g}", [N, R], F8, kind="ExternalInput")
          for g in G}
    # feature-major outputs; host transposes
    out_d = {t: nc.dram_tensor(f"out_{t}", [128, R], F32,
                               kind="ExternalOutput") for t in "AB"}
    dbg_d = {}
    if dbg:
        for t in "AB":
            dbg_d[f"ht_{t}"] = nc.dram_tensor(f"dbg_ht_{t}", [128, R], BF,
                                              kind="ExternalOutput")
        for g in G:
            dbg_d[f"tab_{g}"] = nc.dram_tensor(f"dbg_tab_{g}", [N, 128], BF,
                                               kind="ExternalOutput")

    # internal DRAM for collectives
    tab_loc = {g: nc.dram_tensor(f"tabloc_{g}", [R, 128], BF) for g in G}
    tab = {g: nc.dram_tensor(f"tab_{g}", [N, 128], BF, addr_space="Shared")
           for g in G}

    with tile.TileContext(nc) as tc:
        with (
            tc.tile_pool(name="const", bufs=1) as cp,
            tc.tile_pool(name="big", bufs=1) as bp,
            tc.tile_pool(name="pt", bufs=3) as ptp,
            tc.tile_pool(name="ctp", bufs=3) as ctp,
            tc.tile_pool(name="ps_s", bufs=2, space="PSUM") as ps_s,
            tc.tile_pool(name="ps_u", bufs=2, space="PSUM") as ps_u,
            tc.tile_pool(name="ps_sm", bufs=1, space="PSUM") as ps_sm,
        ):
            # ---------------- constants into SBUF
            def load_const(dram, shape, dt):
                t = cp.tile(shape, dt, tag=dram.name)
                nc.sync.dma_start(out=t[:], in_=dram[:])
                return t

            wqT_s = {t: load_const(wqT[t], [128, 128], BF) for t in "AB"}
            wkT_s = {t: load_const(wkT[t], [128, 128], BF) for t in "AB"}
            wvoT_s = {t: load_const(wvoT[t], [128, 128], BF) for t in "AB"}
            bqs_s = {t: load_const(bqs[t], [128, 1], F32) for t in "AB"}
            bk_s = {t: load_const(bk[t], [128, 1], F32) for t in "AB"}
            wlT_s = {g: load_const(wlT[g], [128, 128], BF) for g in G}
            wrT_s = {t: load_const(wrT[t], [128, 128], BF) for t in "AB"}
            c0_s = {t: load_const(c0[t], [1, 128], F32) for t in "AB"}
            c1_s = {g: load_const(c1[g], [1, 128], F32) for g in G}
            deg_s = {g: load_const(deg[g], [1, R], F32) for g in G}

            ident = cp.tile([128, 128], BF, tag="ident")
            from concourse.masks import make_identity
            make_identity(nc, ident[:])
            ones_col = cp.tile([128, 1], BF, tag="ones_col")
            nc.vector.memset(ones_col[:], 1.0)
            ones_row = cp.tile([1, 512], F32, tag="ones_row")
            nc.vector.memset(ones_row[:], 1.0)

            ht = {t: bp.tile([128, R], BF, tag=f"ht_{t}", name=f"ht_{t}")
                  for t in "AB"}

            # ---------------- attention per type
            for t in "AB":
                x0_s = bp.tile([128, N], BF, tag="x0t")
                nc.sync.dma_start(out=x0_s[:], in_=x0t[t][:])
                x0q_s = bp.tile([128, R], BF, tag="x0q")
                nc.sync.dma_start(out=x0q_s[:], in_=x0q[t][:])

                kt = bp.tile([128, N], BF, tag="kt")
                vt = bp.tile([128, N], BF, tag="vt")
                qt = bp.tile([128, R], BF, tag="qt")

                # Kt = Wk @ x0T (+bk), feature-major
                for j in range(N // 512):
                    ps = ps_s.tile([128, 1024], F32, tag="sc")
                    nc.tensor.matmul(ps[:, :512], lhsT=wkT_s[t][:],
                                     rhs=x0_s[:, j * 512:(j + 1) * 512],
                                     start=True, stop=True)
                    nc.scalar.activation(kt[:, j * 512:(j + 1) * 512],
                                         ps[:, :512],
                                         mybir.ActivationFunctionType.Identity,
                                         bias=bk_s[t][:, :])
                # Qt = s*(Wq @ x0T_own + bq)
                for j in range(R // 512):
                    ps = ps_s.tile([128, 1024], F32, tag="sc")
                    nc.tensor.matmul(ps[:, :512], lhsT=wqT_s[t][:],
                                     rhs=x0q_s[:, j * 512:(j + 1) * 512],
                                     start=True, stop=True)
                    nc.scalar.activation(qt[:, j * 512:(j + 1) * 512],
                                         ps[:, :512],
                                         mybir.ActivationFunctionType.Identity,
                                         bias=bqs_s[t][:, :], scale=float(SCALE))
                # V~ = x0 @ (Wout Wv)^T, node-major blocks (4 per psum tile)
                for vg in range(KB // 4):
                    ps = ps_u.tile([128, 512], F32, tag="ut")
                    for i in range(4):
                        nb = vg * 4 + i
                        nc.tensor.matmul(ps[:, i * 128:(i + 1) * 128],
                                         lhsT=x0_s[:, nb * 128:(nb + 1) * 128],
                                         rhs=wvoT_s[t][:], start=True, stop=True)
                    nc.vector.tensor_copy(vt[:, vg * 512:(vg + 1) * 512], ps[:])

                # attention q-groups
                for qg in range(R // QG):
                    q_sl = slice(qg * QG, (qg + 1) * QG)
                    ut_ps = ps_u.tile([128, QG], F32, tag="ut")
                    racc0 = bp.tile([128, QG], BF, tag="racc0")
                    racc1 = bp.tile([128, QG], BF, tag="racc1")
                    nc.vector.memset(racc0[:], 0.0)
                    nc.vector.memset(racc1[:], 0.0)
                    for pr in range(KB // 2):
                        kb0 = 2 * pr
                        sc = ps_s.tile([128, 1024], F32, tag="sc")
                        nc.tensor.matmul(sc[:, :512],
                                         lhsT=kt[:, kb0 * 128:(kb0 + 1) * 128],
                                         rhs=qt[:, q_sl], start=True, stop=True)
                        nc.tensor.matmul(sc[:, 512:],
                                         lhsT=kt[:, (kb0 + 1) * 128:(kb0 + 2) * 128],
                                         rhs=qt[:, q_sl], start=True, stop=True)
                        pt = ptp.tile([128, 1024], BF, tag="pt")
                        nc.scalar.activation(pt[:], sc[:],
                                             mybir.ActivationFunctionType.Exp)
                        nc.tensor.matmul(ut_ps[:],
                                         lhsT=vt[:, kb0 * 128:(kb0 + 1) * 128],
                                         rhs=pt[:, :512],
                                         start=(pr == 0), stop=False)
                        nc.tensor.matmul(ut_ps[:],
                                         lhsT=vt[:, (kb0 + 1) * 128:(kb0 + 2) * 128],
                                         rhs=pt[:, 512:],
                                         start=False, stop=(pr == KB // 2 - 1))
                        nc.vector.tensor_add(racc0[:], racc0[:], pt[:, :512])
                        nc.vector.tensor_add(racc1[:], racc1[:], pt[:, 512:])

                    ut_sb = bp.tile([128, QG], BF, tag="ut_sb")
                    nc.vector.tensor_copy(ut_sb[:], ut_ps[:])
                    for sub in range(QG // 128):
                        s_sl = slice(sub * 128, (sub + 1) * 128)
                        rp = ps_sm.tile([128, 512], F32, tag="sm")
                        nc.tensor.matmul(rp[:, :1], lhsT=racc0[:, s_sl],
                                         rhs=ones_col[:], start=True, stop=False)
                        nc.tensor.matmul(rp[:, :1], lhsT=racc1[:, s_sl],
                                         rhs=ones_col[:], start=False, stop=True)
                        rinv = bp.tile([128, 1], F32, tag="rinv")
                        nc.vector.reciprocal(rinv[:], rp[:, :1])
                        # transpose Ut tile -> [q, f], scale by 1/r
                        tp = ps_sm.tile([128, 512], BF, tag="smb")
                        nc.tensor.transpose(tp[:, :128], ut_sb[:, s_sl],
                                            ident[:])
                        hn = bp.tile([128, 128], BF, tag="hn")
                        nc.scalar.activation(hn[:], tp[:, :128],
                                             mybir.ActivationFunctionType.Copy,
                                             scale=rinv[:, :])
                        # transpose back -> feature-major ht
                        tp2 = ps_sm.tile([128, 512], BF, tag="smb")
                        nc.tensor.transpose(tp2[:, :128], hn[:], ident[:])
                        nc.vector.tensor_copy(
                            ht[t][:, qg * QG + sub * 128: qg * QG + (sub + 1) * 128],
                            tp2[:, :128])

                # tables with src == t: tab_loc_g = (h @ wl_g^T) node-major
                for g in TABLES_OF[t]:
                    tsb = bp.tile([128, NBLK * 128], BF, tag="tsb",
                                  name=f"tsb_{g}")
                    for nb in range(NBLK):
                        ps = ps_sm.tile([128, 512], F32, tag="sm")
                        nc.tensor.matmul(ps[:, :128],
                                         lhsT=ht[t][:, nb * 128:(nb + 1) * 128],
                                         rhs=wlT_s[g][:], start=True, stop=True)
                        nc.vector.tensor_copy(tsb[:, nb * 128:(nb + 1) * 128],
                                              ps[:, :128])
                    for nb in range(NBLK):
                        nc.sync.dma_start(
                            out=tab_loc[g][nb * 128:(nb + 1) * 128, :],
                            in_=tsb[:, nb * 128:(nb + 1) * 128])
                    nc.gpsimd.collective_compute(
                        "AllGather", mybir.AluOpType.bypass,
                        replica_groups=[list(range(NCORES))],
                        ins=[tab_loc[g].ap()], outs=[tab[g].ap()])

            if dbg:
                for t in "AB":
                    nc.sync.dma_start(out=dbg_d[f"ht_{t}"][:], in_=ht[t][:])
                for g in G:
                    nc.sync.dma_start(out=dbg_d[f"tab_{g}"][:], in_=tab[g][:])

            # ---------------- phase 2: dense count-matrix aggregation
            # out^T[d, dst] = sum_g tab_g^T @ C_g + wr@ht + c0*1 + c1*deg
            for t in ("BA" if stage >= 2 else ""):
                po = []
                for h in range(2):
                    po_t = ps_u.tile([128, 512], F32, tag="ut",
                                     name=f"po_{t}_{h}")
                    po.append(po_t)
                first = [True, True]
                for g in GRAPHS_OF[t]:
                    # table node-major into SBUF: [s within block, blk*128+d]
                    tabsb = bp.tile([128, KB * 128], BF, tag="x0t",
                                    name=f"tabsb_{g}")
                    for sb in range(KB):
                        nc.sync.dma_start(
                            out=tabsb[:, sb * 128:(sb + 1) * 128],
                            in_=tab[g][sb * 128:(sb + 1) * 128, :])
                    for scg in range(8):   # groups of 8 s-blocks
                        ct_t = ctp.tile([128, 8 * R], F8, tag="ct",
                                        name=f"ct_{g}_{scg}")
                        # DRAM [8, 128, R] group -> SBUF [128, 8*R]
                        for sb in range(8):
                            base = scg * 1024 + sb * 128
                            nc.sync.dma_start(
                                out=ct_t[:, sb * R:(sb + 1) * R],
                                in_=ct[g][base:base + 128, :])
                        for sb in range(8):
                            lt = tabsb[:, (scg * 8 + sb) * 128:
                                       (scg * 8 + sb + 1) * 128]
                            for h in range(2):
                                nc.tensor.matmul(
                                    po[h][:],
                                    lhsT=lt,
                                    rhs=ct_t[:, sb * R + h * 512:
                                             sb * R + (h + 1) * 512],
                                    start=first[h], stop=False)
                                first[h] = False
                    # degree correction: out^T += c1 (x) deg
                    for h in range(2):
                        nc.tensor.matmul(po[h][:], lhsT=c1_s[g][:],
                                         rhs=deg_s[g][:, h * 512:(h + 1) * 512],
                                         start=False, stop=False)
                # lin_r:  out^T += wr @ ht
                for h in range(2):
                    nc.tensor.matmul(po[h][:], lhsT=wrT_s[t][:],
                                     rhs=ht[t][:, h * 512:(h + 1) * 512],
                                     start=False, stop=False)
                    # constant c0 per feature
                    nc.tensor.matmul(po[h][:], lhsT=c0_s[t][:],
                                     rhs=ones_row[:], start=False, stop=True)
                    osb = bp.tile([128, 512], F32, tag="osb",
                                  name=f"osb_{t}_{h}")
                    nc.vector.tensor_copy(osb[:], po[h][:])
                    nc.sync.dma_start(out=out_d[t][:, h * 512:(h + 1) * 512],
                                      in_=osb[:])

    nc.compile()
    return nc


# ---------------------------------------------------------------- host prep

def _prep(inputs, dbg=False):
    ins = {k: np.asarray(v) for k, v in inputs.items()}

    def bf(x):
        return np.ascontiguousarray(np.asarray(x, np.float32)).astype(BF16)

    com = {}
    for t in "AB":
        iw = ins[f"inW_{t}"].astype(np.float32)
        ib = ins[f"inB_{t}"].astype(np.float32)
        ow = ins[f"outW_{t}"].astype(np.float32)
        ob = ins[f"outB_{t}"].astype(np.float32)
        com[f"wqT_{t}"] = bf(iw[0:128].T)
        com[f"wkT_{t}"] = bf(iw[128:256].T)
        wvo = ow @ iw[256:384]
        com[f"wvoT_{t}"] = bf(wvo.T)
        com[f"bqs_{t}"] = (ib[0:128] * SCALE).reshape(128, 1).astype(np.float32)
        com[f"bk_{t}"] = ib[128:256].reshape(128, 1).astype(np.float32)
        com[f"bout_eff_{t}"] = ow @ ib[256:384] + ob
    for g in G:
        com[f"wlT_{g}"] = bf(ins[f"wl_{g}"].T)
        com[f"c1_{g}"] = (ins[f"wl_{g}"].astype(np.float32)
                          @ com[f"bout_eff_{SRC_T[g]}"]).reshape(1, 128)
    com["wrT_B"] = bf(ins["wr_AB"].T)
    com["wrT_A"] = bf((ins["wr_BA"] + ins["wr_AA"]).T)
    com["c0_B"] = (ins["bl_AB"].astype(np.float32)
                   + ins["wr_AB"].astype(np.float32) @ com["bout_eff_B"]
                   ).reshape(1, 128)
    com["c0_A"] = (ins["bl_BA"].astype(np.float32)
                   + ins["bl_AA"].astype(np.float32)
                   + (ins["wr_BA"] + ins["wr_AA"]).astype(np.float32)
                   @ com["bout_eff_A"]).reshape(1, 128)

    x0T = {t: np.ascontiguousarray(
        ins[f"x_{t}"][:, 0, :].astype(np.float32).T).astype(BF16)
        for t in "AB"}

    # per-core count matrices [N src, R dst] fp8 + degree vectors
    cts = {}
    degs = {}
    for g in G:
        src = np.asarray(ins[f"ei_{g}"][0], np.int64)
        dst = np.asarray(ins[f"ei_{g}"][1], np.int64)
        per_core = []
        dgs = []
        for c in range(NCORES):
            sel = (dst >> 10) == c
            s_c = src[sel]
            d_c = dst[sel] - c * R
            cmat = np.zeros((N, R), np.float32)
            np.add.at(cmat, (s_c, d_c), 1.0)
            per_core.append(cmat.astype(FP8))
            dgs.append(np.bincount(d_c, minlength=R).astype(np.float32)
                       .reshape(1, R))
        cts[g] = per_core
        degs[g] = dgs

    in_maps = []
    for c in range(NCORES):
        m = {}
        for t in "AB":
            m[f"x0t_{t}"] = x0T[t]
            m[f"x0q_{t}"] = np.ascontiguousarray(x0T[t][:, c * R:(c + 1) * R])
            for k in ("wqT", "wkT", "wvoT", "bqs", "bk", "wrT", "c0"):
                m[f"{k}_{t}"] = com[f"{k}_{t}"]
        for g in G:
            m[f"wlT_{g}"] = com[f"wlT_{g}"]
            m[f"c1_{g}"] = com[f"c1_{g}"].astype(np.float32)
            m[f"deg_{g}"] = degs[g][c]
            m[f"ct_{g}"] = cts[g][c]
        in_maps.append(m)
    return in_maps


def kernel(**inputs):
    in_maps = _prep(inputs)
    if "prog" not in _PROG_CACHE:
        _PROG_CACHE["prog"] = build_program()
    nc = _PROG_CACHE["prog"]
    res = bass_utils.run_bass_kernel_spmd(
        nc, in_maps, core_ids=list(range(NCORES)))
    x_A = np.asarray(inputs["x_A"], np.float32)
    x_B = np.asarray(inputs["x_B"], np.float32)
    new_A = x_A.copy()
    new_B = x_B.copy()
    for c in range(NCORES):
        new_A[c * R:(c + 1) * R, 0, :] = res.results[c]["out_A"].T
        new_B[c * R:(c + 1) * R, 0, :] = res.results[c]["out_B"].T
    return new_A, new_B


# revision 22
# speedup vs baseline: 4.3202x; 1.1261x over previous
"""Trainium2 Bass kernel for nn_AttnDBGNNLayer (8-core SPMD).

kernel(**inputs) takes the FULL inputs (as produced by setup_inputs) and
returns the FULL output (new_A, new_B), distributing across 8 NeuronCores.

Design:
- q-rows of both attentions sharded 8-way (1024 rows/core); K/V computed
  replicated from a feature-major x0^T; single-pass unnormalized softmax
  (scores are tiny, no max subtraction); out-projection folded into V
  (Wvo = Wout @ Wv); row-sum via DVE accumulation + ones-matmul; normalize
  via PE-transpose + per-partition scale.
- per-graph gather tables hold h @ wl^T (lin_l folded), AllGathered in bf16.
- message aggregation as dense count-matrix matmuls: out^T += tab_g^T @ C_g
  with C_g the per-core [8192 src, 1024 dst] edge-count matrix in fp8
  (counts are small ints, exact); lin_r / biases / degree corrections are
  folded in as K=1 matmuls into the same PSUM accumulation.
- outputs are produced feature-major and transposed on the host.
"""
import sys

if "/opt/trn_rl_repo" not in sys.path:
    sys.path.insert(0, "/opt/trn_rl_repo")

import numpy as np
import ml_dtypes

import concourse.bacc as bacc
import concourse.tile as tile
import concourse.mybir as mybir
from concourse import bass_utils

BF16 = ml_dtypes.bfloat16
FP8 = ml_dtypes.float8_e4m3

N = 8192          # nodes per type
D = 128           # feature dim
NCORES = 8
R = N // NCORES   # rows (q / dst nodes) per core = 1024
QG = 512          # q-group width
KB = N // 128     # number of 128-wide k blocks = 64
NBLK = R // 128   # dst blocks per core = 8
SCALE = 1.0 / np.sqrt(np.float32(D))

F32 = mybir.dt.float32
BF = mybir.dt.bfloat16
F8 = mybir.dt.float8e4

G = ("AB", "BA", "AA")
SRC_T = {"AB": "A", "BA": "B", "AA": "A"}
GRAPHS_OF = {"A": ("BA", "AA"), "B": ("AB",)}   # inbound graphs per output
TABLES_OF = {"A": ("AB", "AA"), "B": ("BA",)}   # tables whose src is t

_PROG_CACHE = {}


def build_program(dbg=False, stage=3):
    nc = bacc.Bacc("TRN2", target_bir_lowering=False, debug=False,
                   num_devices=NCORES)

    # ---- I/O (identical shapes on all cores; per-core data)
    x0t = {t: nc.dram_tensor(f"x0t_{t}", [128, N], BF, kind="ExternalInput")
           for t in "AB"}
    x0q = {t: nc.dram_tensor(f"x0q_{t}", [128, R], BF, kind="ExternalInput")
           for t in "AB"}
    wqT = {t: nc.dram_tensor(f"wqT_{t}", [128, 128], BF, kind="ExternalInput")
           for t in "AB"}
    wkT = {t: nc.dram_tensor(f"wkT_{t}", [128, 128], BF, kind="ExternalInput")
           for t in "AB"}
    wvoT = {t: nc.dram_tensor(f"wvoT_{t}", [128, 128], BF, kind="ExternalInput")
            for t in "AB"}
    bqs = {t: nc.dram_tensor(f"bqs_{t}", [128, 1], F32, kind="ExternalInput")
           for t in "AB"}
    bk = {t: nc.dram_tensor(f"bk_{t}", [128, 1], F32, kind="ExternalInput")
          for t in "AB"}
    wlT = {g: nc.dram_tensor(f"wlT_{g}", [128, 128], BF, kind="ExternalInput")
           for g in G}
    wrT = {t: nc.dram_tensor(f"wrT_{t}", [128, 128], BF, kind="ExternalInput")
           for t in "AB"}
    c0 = {t: nc.dram_tensor(f"c0_{t}", [1, 128], F32, kind="ExternalInput")
          for t in "AB"}
    c1 = {g: nc.dram_tensor(f"c1_{g}", [1, 128], F32, kind="ExternalInput")
          for g in G}
    deg = {g: nc.dram_tensor(f"deg_{g}", [1, R], F32, kind="ExternalInput")
           for g in G}
    ct = {g: nc.dram_tensor(f"ct_{g}", [N, R], F8, kind="ExternalInput")
          for g in G}
    # feature-major outputs; host transposes
    out_d = {t: nc.dram_tensor(f"out_{t}", [128, R], F32,
                               kind="ExternalOutput") for t in "AB"}
    dbg_d = {}
    if dbg:
        for t in "AB":
            dbg_d[f"ht_{t}"] = nc.dram_tensor(f"dbg_ht_{t}", [128, R], BF,
                                              kind="ExternalOutput")
        for g in G:
            dbg_d[f"tab_{g}"] = nc.dram_tensor(f"dbg_tab_{g}", [N, 128], BF,
                                               kind="ExternalOutput")

    # internal DRAM for collectives
    tab_loc = {g: nc.dram_tensor(f"tabloc_{g}", [R, 128], BF) for g in G}
    tab = {g: nc.dram_tensor(f"tab_{g}", [N, 128], BF, addr_space="Shared")
           for g in G}

    with tile.TileContext(nc) as tc:
        with (
            tc.tile_pool(name="const", bufs=1) as cp,
            tc.tile_pool(name="big", bufs=1) as bp,
            tc.tile_pool(name="pt", bufs=3) as ptp,
            tc.tile_pool(name="ctp", bufs=3) as ctp,
            tc.tile_pool(name="ps_s", bufs=2, space="PSUM") as ps_s,
            tc.tile_pool(name="ps_u", bufs=2, space="PSUM") as ps_u,
            tc.tile_pool(name="ps_sm", bufs=2, space="PSUM") as ps_sm,
        ):
            # ---------------- constants into SBUF
            def load_const(dram, shape, dt):
                t = cp.tile(shape, dt, tag=dram.name)
                nc.sync.dma_start(out=t[:], in_=dram[:])
                return t

            wqT_s = {t: load_const(wqT[t], [128, 128], BF) for t in "AB"}
            wkT_s = {t: load_const(wkT[t], [128, 128], BF) for t in "AB"}
            wvoT_s = {t: load_const(wvoT[t], [128, 128], BF) for t in "AB"}
            bqs_s = {t: load_const(bqs[t], [128, 1], F32) for t in "AB"}
            bk_s = {t: load_const(bk[t], [128, 1], F32) for t in "AB"}
            wlT_s = {g: load_const(wlT[g], [128, 128], BF) for g in G}
            wrT_s = {t: load_const(wrT[t], [128, 128], BF) for t in "AB"}
            c0_s = {t: load_const(c0[t], [1, 128], F32) for t in "AB"}
            c1_s = {g: load_const(c1[g], [1, 128], F32) for g in G}
            deg_s = {g: load_const(deg[g], [1, R], F32) for g in G}

            ident = cp.tile([128, 128], BF, tag="ident")
            from concourse.masks import make_identity
            make_identity(nc, ident[:])
            ones_col = cp.tile([128, 1], BF, tag="ones_col")
            nc.vector.memset(ones_col[:], 1.0)
            ones_row = cp.tile([1, 512], F32, tag="ones_row")
            nc.vector.memset(ones_row[:], 1.0)

            ht = {t: bp.tile([128, R], BF, tag=f"ht_{t}", name=f"ht_{t}")
                  for t in "AB"}

            # ---------------- attention per type
            for t in "AB":
                x0_s = bp.tile([128, N], BF, tag="x0t")
                nc.sync.dma_start(out=x0_s[:], in_=x0t[t][:])
                x0q_s = bp.tile([128, R], BF, tag="x0q")
                nc.sync.dma_start(out=x0q_s[:], in_=x0q[t][:])

                kt = bp.tile([128, N], BF, tag="kt")
                vt = bp.tile([128, N], BF, tag="vt")
                qt = bp.tile([128, R], BF, tag="qt")

                # Kt = Wk @ x0T (+bk), feature-major
                for j in range(N // 512):
                    ps = ps_s.tile([128, 1024], F32, tag="sc")
                    nc.tensor.matmul(ps[:, :512], lhsT=wkT_s[t][:],
                                     rhs=x0_s[:, j * 512:(j + 1) * 512],
                                     start=True, stop=True)
                    nc.scalar.activation(kt[:, j * 512:(j + 1) * 512],
                                         ps[:, :512],
                                         mybir.ActivationFunctionType.Identity,
                                         bias=bk_s[t][:, :])
                # Qt = s*(Wq @ x0T_own + bq)
                for j in range(R // 512):
                    ps = ps_s.tile([128, 1024], F32, tag="sc")
                    nc.tensor.matmul(ps[:, :512], lhsT=wqT_s[t][:],
                                     rhs=x0q_s[:, j * 512:(j + 1) * 512],
                                     start=True, stop=True)
                    nc.scalar.activation(qt[:, j * 512:(j + 1) * 512],
                                         ps[:, :512],
                                         mybir.ActivationFunctionType.Identity,
                                         bias=bqs_s[t][:, :], scale=float(SCALE))
                # V~ = x0 @ (Wout Wv)^T, node-major blocks (4 per psum tile)
                for vg in range(KB // 4):
                    ps = ps_u.tile([128, 512], F32, tag="ut")
                    for i in range(4):
                        nb = vg * 4 + i
                        nc.tensor.matmul(ps[:, i * 128:(i + 1) * 128],
                                         lhsT=x0_s[:, nb * 128:(nb + 1) * 128],
                                         rhs=wvoT_s[t][:], start=True, stop=True)
                    nc.vector.tensor_copy(vt[:, vg * 512:(vg + 1) * 512], ps[:])

                # attention q-groups
                for qg in range(R // QG):
                    q_sl = slice(qg * QG, (qg + 1) * QG)
                    ut_ps = ps_u.tile([128, QG], F32, tag="ut")
                    racc0 = bp.tile([128, QG], BF, tag="racc0")
                    racc1 = bp.tile([128, QG], BF, tag="racc1")
                    nc.vector.memset(racc0[:], 0.0)
                    nc.vector.memset(racc1[:], 0.0)
                    for pr in range(KB // 2):
                        kb0 = 2 * pr
                        sc = ps_s.tile([128, 1024], F32, tag="sc")
                        nc.tensor.matmul(sc[:, :512],
                                         lhsT=kt[:, kb0 * 128:(kb0 + 1) * 128],
                                         rhs=qt[:, q_sl], start=True, stop=True)
                        nc.tensor.matmul(sc[:, 512:],
                                         lhsT=kt[:, (kb0 + 1) * 128:(kb0 + 2) * 128],
                                         rhs=qt[:, q_sl], start=True, stop=True)
                        pt = ptp.tile([128, 1024], BF, tag="pt")
                        nc.scalar.activation(pt[:], sc[:],
                                             mybir.ActivationFunctionType.Exp)
                        nc.tensor.matmul(ut_ps[:],
                                         lhsT=vt[:, kb0 * 128:(kb0 + 1) * 128],
                                         rhs=pt[:, :512],
                                         start=(pr == 0), stop=False)
                        nc.tensor.matmul(ut_ps[:],
                                         lhsT=vt[:, (kb0 + 1) * 128:(kb0 + 2) * 128],
                                         rhs=pt[:, 512:],
                                         start=False, stop=(pr == KB // 2 - 1))
                        nc.vector.tensor_add(racc0[:], racc0[:], pt[:, :512])
                        nc.vector.tensor_add(racc1[:], racc1[:], pt[:, 512:])

                    ut_sb = bp.tile([128, QG], BF, tag="ut_sb")
                    nc.vector.tensor_copy(ut_sb[:], ut_ps[:])
                    for sub in range(QG // 128):
                        s_sl = slice(sub * 128, (sub + 1) * 128)
                        rp = ps_sm.tile([128, 512], F32, tag="sm")
                        nc.tensor.matmul(rp[:, :1], lhsT=racc0[:, s_sl],
                                         rhs=ones_col[:], start=True, stop=False)
                        nc.tensor.matmul(rp[:, :1], lhsT=racc1[:, s_sl],
                                         rhs=ones_col[:], start=False, stop=True)
                        rinv = bp.tile([128, 1], F32, tag="rinv")
                        nc.vector.reciprocal(rinv[:], rp[:, :1])
                        # transpose Ut tile -> [q, f], scale by 1/r
                        tp = ps_sm.tile([128, 512], BF, tag="sm", name="tp")
                        nc.tensor.transpose(tp[:, :128], ut_sb[:, s_sl],
                                            ident[:])
                        hn = bp.tile([128, 128], BF, tag="hn")
                        nc.scalar.activation(hn[:], tp[:, :128],
                                             mybir.ActivationFunctionType.Copy,
                                             scale=rinv[:, :])
                        # transpose back -> feature-major ht
                        tp2 = ps_sm.tile([128, 512], BF, tag="sm", name="tp2")
                        nc.tensor.transpose(tp2[:, :128], hn[:], ident[:])
                        nc.vector.tensor_copy(
                            ht[t][:, qg * QG + sub * 128: qg * QG + (sub + 1) * 128],
                            tp2[:, :128])

                # tables with src == t: tab_loc_g = (h @ wl_g^T) node-major
                for g in TABLES_OF[t]:
                    tsb = bp.tile([128, NBLK * 128], BF, tag="tsb",
                                  name=f"tsb_{g}")
                    for nb in range(NBLK):
                        ps = ps_sm.tile([128, 512], F32, tag="sm")
                        nc.tensor.matmul(ps[:, :128],
                                         lhsT=ht[t][:, nb * 128:(nb + 1) * 128],
                                         rhs=wlT_s[g][:], start=True, stop=True)
                        nc.vector.tensor_copy(tsb[:, nb * 128:(nb + 1) * 128],
                                              ps[:, :128])
                    for nb in range(NBLK):
                        nc.sync.dma_start(
                            out=tab_loc[g][nb * 128:(nb + 1) * 128, :],
                            in_=tsb[:, nb * 128:(nb + 1) * 128])
                    nc.gpsimd.collective_compute(
                        "AllGather", mybir.AluOpType.bypass,
                        replica_groups=[list(range(NCORES))],
                        ins=[tab_loc[g].ap()], outs=[tab[g].ap()])

            if dbg:
                for t in "AB":
                    nc.sync.dma_start(out=dbg_d[f"ht_{t}"][:], in_=ht[t][:])
                for g in G:
                    nc.sync.dma_start(out=dbg_d[f"tab_{g}"][:], in_=tab[g][:])

            # ---------------- phase 2: dense count-matrix aggregation
            # out^T[d, dst] = sum_g tab_g^T @ C_g + wr@ht + c0*1 + c1*deg
            for t in ("BA" if stage >= 2 else ""):
                po = []
                for h in range(2):
                    po_t = ps_u.tile([128, 512], F32, tag="ut",
                                     name=f"po_{t}_{h}")
                    po.append(po_t)
                first = [True, True]
                for g in GRAPHS_OF[t]:
                    # table node-major into SBUF: [s within block, blk*128+d]
                    tabsb = bp.tile([128, KB * 128], BF, tag="x0t",
                                    name=f"tabsb_{g}")
                    nc.sync.dma_start(
                        out=tabsb[:].rearrange("s (b d) -> s b d", d=128),
                        in_=tab[g][:].rearrange("(b s) d -> s b d", s=128))
                    for scg in range(8):   # groups of 8 s-blocks
                        ct_t = ctp.tile([128, 8 * R], F8, tag="ct",
                                        name=f"ct_{g}_{scg}")
                        nc.sync.dma_start(
                            out=ct_t[:].rearrange("s (b d) -> s b d", d=R),
                            in_=ct[g][scg * 1024:(scg + 1) * 1024, :]
                            .rearrange("(b s) d -> s b d", s=128))
                        for sb in range(8):
                            lt = tabsb[:, (scg * 8 + sb) * 128:
                                       (scg * 8 + sb + 1) * 128]
                            for h in range(2):
                                nc.tensor.matmul(
                                    po[h][:],
                                    lhsT=lt,
                                    rhs=ct_t[:, sb * R + h * 512:
                                             sb * R + (h + 1) * 512],
                                    start=first[h], stop=False)
                                first[h] = False
                    # degree correction: out^T += c1 (x) deg
                    for h in range(2):
                        nc.tensor.matmul(po[h][:], lhsT=c1_s[g][:],
                                         rhs=deg_s[g][:, h * 512:(h + 1) * 512],
                                         start=False, stop=False)
                # lin_r:  out^T += wr @ ht
                for h in range(2):
                    nc.tensor.matmul(po[h][:], lhsT=wrT_s[t][:],
                                     rhs=ht[t][:, h * 512:(h + 1) * 512],
                                     start=False, stop=False)
                    # constant c0 per feature
                    nc.tensor.matmul(po[h][:], lhsT=c0_s[t][:],
                                     rhs=ones_row[:], start=False, stop=True)
                    osb = bp.tile([128, 512], F32, tag="osb",
                                  name=f"osb_{t}_{h}")
                    nc.vector.tensor_copy(osb[:], po[h][:])
                    nc.sync.dma_start(out=out_d[t][:, h * 512:(h + 1) * 512],
                                      in_=osb[:])

    nc.compile()
    return nc


# ---------------------------------------------------------------- host prep

def _prep(inputs, dbg=False):
    ins = {k: np.asarray(v) for k, v in inputs.items()}

    def bf(x):
        return np.ascontiguousarray(np.asarray(x, np.float32)).astype(BF16)

    com = {}
    for t in "AB":
        iw = ins[f"inW_{t}"].astype(np.float32)
        ib = ins[f"inB_{t}"].astype(np.float32)
        ow = ins[f"outW_{t}"].astype(np.float32)
        ob = ins[f"outB_{t}"].astype(np.float32)
        com[f"wqT_{t}"] = bf(iw[0:128].T)
        com[f"wkT_{t}"] = bf(iw[128:256].T)
        wvo = ow @ iw[256:384]
        com[f"wvoT_{t}"] = bf(wvo.T)
        com[f"bqs_{t}"] = (ib[0:128] * SCALE).reshape(128, 1).astype(np.float32)
        com[f"bk_{t}"] = ib[128:256].reshape(128, 1).astype(np.float32)
        com[f"bout_eff_{t}"] = ow @ ib[256:384] + ob
    for g in G:
        com[f"wlT_{g}"] = bf(ins[f"wl_{g}"].T)
        com[f"c1_{g}"] = (ins[f"wl_{g}"].astype(np.float32)
                          @ com[f"bout_eff_{SRC_T[g]}"]).reshape(1, 128)
    com["wrT_B"] = bf(ins["wr_AB"].T)
    com["wrT_A"] = bf((ins["wr_BA"] + ins["wr_AA"]).T)
    com["c0_B"] = (ins["bl_AB"].astype(np.float32)
                   + ins["wr_AB"].astype(np.float32) @ com["bout_eff_B"]
                   ).reshape(1, 128)
    com["c0_A"] = (ins["bl_BA"].astype(np.float32)
                   + ins["bl_AA"].astype(np.float32)
                   + (ins["wr_BA"] + ins["wr_AA"]).astype(np.float32)
                   @ com["bout_eff_A"]).reshape(1, 128)

    x0T = {t: np.ascontiguousarray(
        ins[f"x_{t}"][:, 0, :].astype(np.float32).T).astype(BF16)
        for t in "AB"}

    # per-core count matrices [N src, R dst] fp8 + degree vectors
    cts = {}
    degs = {}
    for g in G:
        src = np.asarray(ins[f"ei_{g}"][0], np.int64)
        dst = np.asarray(ins[f"ei_{g}"][1], np.int64)
        per_core = []
        dgs = []
        for c in range(NCORES):
            sel = (dst >> 10) == c
            s_c = src[sel]
            d_c = dst[sel] - c * R
            cmat = np.zeros((N, R), np.float32)
            np.add.at(cmat, (s_c, d_c), 1.0)
            per_core.append(cmat.astype(FP8))
            dgs.append(np.bincount(d_c, minlength=R).astype(np.float32)
                       .reshape(1, R))
        cts[g] = per_core
        degs[g] = dgs

    in_maps = []
    for c in range(NCORES):
        m = {}
        for t in "AB":
            m[f"x0t_{t}"] = x0T[t]
            m[f"x0q_{t}"] = np.ascontiguousarray(x0T[t][:, c * R:(c + 1) * R])
            for k in ("wqT", "wkT", "wvoT", "bqs", "bk", "wrT", "c0"):
                m[f"{k}_{t}"] = com[f"{k}_{t}"]
        for g in G:
            m[f"wlT_{g}"] = com[f"wlT_{g}"]
            m[f"c1_{g}"] = com[f"c1_{g}"].astype(np.float32)
            m[f"deg_{g}"] = degs[g][c]
            m[f"ct_{g}"] = cts[g][c]
        in_maps.append(m)
    return in_maps


def kernel(**inputs):
    in_maps = _prep(inputs)
    if "prog" not in _PROG_CACHE:
        _PROG_CACHE["prog"] = build_program()
    nc = _PROG_CACHE["prog"]
    res = bass_utils.run_bass_kernel_spmd(
        nc, in_maps, core_ids=list(range(NCORES)))
    x_A = np.asarray(inputs["x_A"], np.float32)
    x_B = np.asarray(inputs["x_B"], np.float32)
    new_A = x_A.copy()
    new_B = x_B.copy()
    for c in range(NCORES):
        new_A[c * R:(c + 1) * R, 0, :] = res.results[c]["out_A"].T
        new_B[c * R:(c + 1) * R, 0, :] = res.results[c]["out_B"].T
    return new_A, new_B
